# revision 4
# baseline (speedup 1.0000x reference)
"""Trainium2 Bass kernel for nn_AdversarialFeatureDropout — bf16, 1-op,
graduated chunks.

out[b, s, f] = x'[b, s, f] * M[b, f]   (x' = host-patched mimic rows,
M = 0/1 drop mask; see kernel3). Device stream in bf16.

Schedule refinements over kernel3:
  - graduated seq-chunks: small tiles at the global start (faster
    pipeline fill -> stores start sooner) and at the global end
    (shorter load->mult->store drain chain), 32-seq tiles in the body.
  - M control tiles ride the ACT HWDGE ring (empty at kernel start)
    so they don't queue behind the first x loads on the SP ring.
"""

import numpy as np
import ml_dtypes

B, S, F = 2048, 128, 256
N_DROP = 200
P_SINGLE, P_DOUBLE, P_MIMIC = 0.3, 0.15, 0.1
NCORES = 8
BSH = B // NCORES  # 256 samples per core
BLK = 128          # partition block (samples)

# per-block seq-chunk schedules (sum = S = 128 each)
CHUNKS0 = [8, 8, 16, 32, 32, 32]   # ramp-up at global start
CHUNKS1 = [32, 32, 32, 16, 8, 8]   # ramp-down at global end

_NC_CACHE = {}

BF16 = ml_dtypes.bfloat16


def _to_bf16_rne(x32):
    """fp32 -> bf16 round-to-nearest-even, vectorized (no NaN in data)."""
    u = np.ascontiguousarray(x32, dtype=np.float32).view(np.uint32)
    r = ((u + np.uint32(0x7FFF) + ((u >> np.uint32(16)) & np.uint32(1)))
         >> np.uint32(16)).astype(np.uint16)
    return r.view(BF16)


def _build_nc():
    import concourse.bass as bass
    import concourse.bacc as bacc
    import concourse.mybir as mybir
    import concourse.tile as tile

    nc = bacc.Bacc("TRN2", target_bir_lowering=False, debug=False,
                   num_devices=NCORES)
    bf16 = mybir.dt.bfloat16
    x_t = nc.dram_tensor("x", [BSH, S, F], bf16, kind="ExternalInput")
    m_t = nc.dram_tensor("m", [BSH, F], bf16, kind="ExternalInput")
    o_t = nc.dram_tensor("o", [BSH, S, F], bf16, kind="ExternalOutput")

    x2 = x_t.ap().rearrange("b s f -> b (s f)")
    o2 = o_t.ap().rearrange("b s f -> b (s f)")
    m2 = m_t.ap()

    with tile.TileContext(nc) as tc:
        with tc.tile_pool(name="xp", bufs=8) as xp, \
             tc.tile_pool(name="cp", bufs=2) as cp:
            for blk, chunks in enumerate((CHUNKS0, CHUNKS1)):
                rows = slice(blk * BLK, (blk + 1) * BLK)
                mt = cp.tile([BLK, F], bf16, tag="mt")
                nc.scalar.dma_start(mt, m2[rows, :])
                mb = mt.rearrange("p (o f) -> p o f", o=1)
                s0 = 0
                for sch in chunks:
                    cols = slice(s0 * F, (s0 + sch) * F)
                    s0 += sch
                    xt = xp.tile([BLK, sch * F], bf16, tag="xt")
                    nc.sync.dma_start(xt, x2[rows, cols])
                    xv = xt.rearrange("p (s f) -> p s f", f=F)
                    mbb, _ = bass.broadcast_tensor_aps(mb, xv)
                    nc.vector.tensor_tensor(xv, xv, mbb, mybir.AluOpType.mult)
                    nc.scalar.dma_start(o2[rows, cols], xt)
    nc.compile()
    return nc


def get_nc():
    if "nc" not in _NC_CACHE:
        _NC_CACHE["nc"] = _build_nc()
    return _NC_CACHE["nc"]


def make_in_maps(x, benign_means, r, y, perm2, feat_mimic):
    benign_means = np.asarray(benign_means, dtype=np.float32)
    r = np.asarray(r, dtype=np.float32)
    y = np.asarray(y)
    perm2 = np.asarray(perm2)
    feat_mimic = np.asarray(feat_mimic)

    t_drop = np.float32(P_SINGLE + P_DOUBLE)
    t_two = np.float32(P_DOUBLE)
    t_mim = np.float32(P_SINGLE + P_DOUBLE + P_MIMIC)
    drop_any = r < t_drop
    drop_two = r < t_two
    mimic = (r >= t_drop) & (r < t_mim) & (y < benign_means.shape[0])
    bidx = np.arange(r.shape[0])

    M = np.ones((r.shape[0], F), np.float32)
    M[bidx[drop_any], perm2[drop_any, 0]] = 0.0
    M[bidx[drop_two], perm2[drop_two, 1]] = 0.0
    Mb = M.astype(BF16)

    xb = _to_bf16_rne(np.asarray(x))
    mrows = bidx[mimic]
    fm = feat_mimic[mimic]
    vals = _to_bf16_rne(benign_means[y[mimic], fm])
    xb[mrows[:, None], np.arange(S)[None, :], fm[:, None]] = vals[:, None]

    return [
        {
            "x": xb[i * BSH:(i + 1) * BSH],
            "m": Mb[i * BSH:(i + 1) * BSH],
        }
        for i in range(NCORES)
    ]


def _ensure_ntff_hook_module():
    """concourse.bass_utils does an unguarded `from antenv.axon_hooks
    import get_axon_ntff_profile_hook` when tracing is requested (e.g.
    BASS_TRACE=1). Some images lack that submodule; provide one so a
    traced run degrades gracefully (and profiles via libaxon_pjrt.so
    when the ctypes symbols exist)."""
    import sys
    import types
    try:
        import antenv.axon_hooks  # noqa: F401
        return
    except ImportError:
        pass
    try:
        import antenv
    except ImportError:
        return
    hook = [None]
    try:
        sys.path.insert(0, "/root/.axon_site")
        from trn_agent_boot.trn_boot import _ntff_profile_via_ctypes
        hook[0] = _ntff_profile_via_ctypes("/opt/axon/libaxon_pjrt.so")
    except Exception:
        hook[0] = None
    mod = types.ModuleType("antenv.axon_hooks")
    mod.get_axon_ntff_profile_hook = lambda: hook[0]
    mod.set_axon_ntff_profile_hook = lambda h: hook.__setitem__(0, h)
    sys.modules["antenv.axon_hooks"] = mod
    antenv.axon_hooks = mod


def kernel(x, benign_means, r, y, perm2, feat_mimic):
    import os
    if os.environ.get("BASS_TRACE"):
        _ensure_ntff_hook_module()
    from concourse.bass_utils import run_bass_kernel_spmd

    in_maps = make_in_maps(x, benign_means, r, y, perm2, feat_mimic)
    res = run_bass_kernel_spmd(get_nc(), in_maps, core_ids=list(range(NCORES)))
    out = np.concatenate([res.results[i]["o"] for i in range(NCORES)], axis=0)
    return out.astype(np.float32)
